# revision 48
# baseline (speedup 1.0000x reference)
"""AttentionSequencePoolingLayer Trainium2 kernel (8-core data parallel), v3.

B=2048, S=200, D=64, H1=64, H2=16. Batch sharded 256/core.

Strategy:
- Rows are globally sorted by seq_length and dealt round-robin to the 8 cores,
  so every core sees the same length profile. Within a core, rows are grouped
  16 at a time; group g only processes T_g = roundup(max seq_length, 16)
  tokens (ΣT ≈ 0.57 × S). One program (compiled per schedule) serves all cores.
- k is re-laid token-major per 64-row block on the host (8KB DMA descriptors,
  zero-padded to 256 tokens); cast-load bf16 (tokens on partitions), xbar-transpose
  to kT [(bhat,d), tok] pair tiles.
- Groups are CLUSTERED into 3-PSUM-bank z1 super-tiles (first-fit chunk packing,
  dead columns allowed and simply processed as garbage): one sigmoid + one
  fused dice op per cluster instead of per ~64-token unit.
- z1 = x1 = att@W1 in PSUM via 3 accumulating matmuls per pair chunk:
  qW ⊗ ones (K=1 rank-1), Wk^T kT, (q∘Wqk)^T kT.
- dice via scalar_tensor_tensor with (1-alpha) folded into the next weights:
  dice(x) = (1-a)*x*(p + a/(1-a)); w2na = (1-a1)W2, w34 = (1-a2)W3.
  Route r1: DVE stt reads z1 PSUM directly. Route r2b (rare): ACT copy
  x1->SBUF, DVE 2x tensor_tensor, z2 = w2a^T x1 + w2na^T u1.
- z2 + scores + pooling PSUM share one bank per cluster (z2 region | per-group
  48-col score/pool regions); dice2 = one sigmoid + one stt per cluster.
- scores via tiny N=2 matmuls into token-major PSUM; σ+mask; pooling as
  out[d,1] = k_nat^T w with N=1 matmuls (k stationary); d-major DRAM output,
  transposed on host at gather.
- Emission is software-pipelined: cluster c's tail (sg/wt/pooling/po-copy) is
  emitted after cluster c+1's head, so in-order engine queues don't
  head-of-line block the next cluster's big ops.
"""
import numpy as np
import ml_dtypes

import concourse.bacc as bacc
import concourse.tile as tile
import concourse.mybir as mybir
import concourse.bass as bass
from concourse.bass_utils import run_bass_kernel_spmd

B, S, D = 2048, 200, 64
H1, H2 = 64, 16
EPS = 1e-9
NCORES = 8
BLOC = B // NCORES          # 256 rows per core
NGROUPS = BLOC // 16        # 16

F32 = mybir.dt.float32
BF16 = mybir.dt.bfloat16
AF = mybir.ActivationFunctionType
ALU = mybir.AluOpType
bf = ml_dtypes.bfloat16

# fraction of clusters on route-2b (ACT-heavy) vs route-1 (DVE-heavy): num/den
R2B_NUM, R2B_DEN = 1, 8
R2B_PAT = ""
LP_BUFS, KT_BUFS, WP_BUFS, H2_BUFS = 3, 6, 6, 3
PS1_BUFS, PS2_BUFS = 2, 4
Z1_BANKS = 2            # banks per z1 super-tile (512 f32 cols each)
TAIL_DEFER = True
WT_POOL = False
CSPLIT = False
PSUM_INIT = "none"       # 'none' | 'act' | 'dve' one-time PSUM buf init
SPLIT_GE = 99           # groups >= this index are force-split into pair-halves
WQ_POOL = ()            # pair indices j whose q*Wqk build runs on Pool
H2ACT_PAT = ""          # per-cluster '1' = dice2 via ACT copy + 2x DVE mult
LOAD_BLOCKS = ((0, 4), (4, 4), (8, 4), (12, 4))  # (start_group, ngroups)

_CACHE = {}
TRACE = False
LAST_RESULT = None


def _ceil16(x):
    return (int(x) + 15) // 16 * 16


def _pack_clusters(sched):
    """Partition groups into clusters; pack each pair-chunk (T1/T2 separately)
    into 512-col banks of a Z1_BANKS-bank z1 super-tile, never crossing a bank
    boundary. Returns list of clusters:
      {groups: [g..], span: int, chunks: {(g, j, part): col}, z2base: {g: col},
       c2span: int, scbase: {g: col}}
    Clusters never cross 4-group load-block boundaries. A group whose chunks
    do not fit in an empty tile is split into two pair-halves.
    """
    clusters = []

    def new_cluster():
        return {"groups": [], "banks": [0] * Z1_BANKS, "chunks": {},
                "span": 0, "z2base": {}, "c2span": 0, "scbase": {}}

    def try_place(cl, g, pairs, record=True):
        T1, T2 = sched[g]
        banks = list(cl["banks"])
        place = {}
        for part, T in ((0, T1), (1, T2)):
            if T == 0:
                continue
            for j in pairs:
                for b in range(Z1_BANKS):
                    if banks[b] + T <= 512:
                        place[(g, j, part)] = 512 * b + banks[b]
                        banks[b] += T
                        break
                else:
                    return False
        if record:
            cl["banks"] = banks
            cl["chunks"].update(place)
            cl["span"] = max(512 * b + banks[b] for b in range(Z1_BANKS)
                             if banks[b] > 0)
        return True

    g = 0
    while g < NGROUPS:
        T1, T2 = sched[g]
        Teff = T1 + T2
        cl = clusters[-1] if clusters else None
        # C2/sc budget: z2 region + 48 cols per group must fit one 512-col bank
        def fits_c2(cl_, add_groups):
            ng = len(cl_["groups"]) + add_groups if cl_ else add_groups
            c2 = (cl_["c2span"] if cl_ else 0) + 2 * Teff * add_groups
            return c2 + 48 * ng <= 512
        if (cl is not None and g % 4 != 0 and len(cl["groups"]) > 0
                and fits_c2(cl, 1) and try_place(cl, g, range(8), record=False)
                and cl["groups"][0] // 4 == g // 4):
            try_place(cl, g, range(8))
            cl["z2base"][g] = cl["c2span"]
            cl["c2span"] += 2 * Teff
            cl["groups"].append(g)
        else:
            cl = new_cluster()
            clusters.append(cl)
            if g < SPLIT_GE and try_place(cl, g, range(8)):
                cl["z2base"][g] = 0
                cl["c2span"] = 2 * Teff
                cl["groups"].append(g)
            else:
                # split group into two half-pair clusters
                clusters.pop()
                for half in (range(0, 4), range(4, 8)):
                    cl = new_cluster()
                    clusters.append(cl)
                    assert try_place(cl, g, half)
                    cl["z2base"][g] = 0
                    cl["c2span"] = 2 * Teff  # 4 pairs * 2 slots... see below
                    cl["groups"].append(g)
                    cl["pairs"] = tuple(half)
        g += 1

    for cl in clusters:
        ng = len(cl["groups"])
        for i, gg in enumerate(cl["groups"]):
            cl["scbase"][gg] = cl["c2span"] + 48 * i
        cl.setdefault("pairs", tuple(range(8)))
    return clusters


def _build(sched):
    nc = bacc.Bacc("TRN2", target_bir_lowering=False, debug=False, num_devices=NCORES,
                   dynamic_dma_scratch_size=16384)
    nb = BLOC
    npair = nb // 2  # 128

    ST = 256  # padded tokens per load block (two 128-token chunks)
    keyt = nc.dram_tensor("keyt", [4 * ST * 64, D], F32, kind="ExternalInput").ap()
    qp = nc.dram_tensor("qp", [128, npair], F32, kind="ExternalInput").ap()
    qw1 = nc.dram_tensor("qw1", [1, 128 * npair], BF16, kind="ExternalInput").ap()
    maskd = nc.dram_tensor("maskd", [128, 32 * NGROUPS], BF16, kind="ExternalInput").ap()
    wk2 = nc.dram_tensor("wk2", [128, 128], BF16, kind="ExternalInput").ap()
    wqk2 = nc.dram_tensor("wqk2", [128, 128], BF16, kind="ExternalInput").ap()
    w2a = nc.dram_tensor("w2a", [128, 32], BF16, kind="ExternalInput").ap()
    w2na = nc.dram_tensor("w2na", [128, 32], BF16, kind="ExternalInput").ap()
    w34 = nc.dram_tensor("w34", [128, 2], BF16, kind="ExternalInput").ap()
    colsb = nc.dram_tensor("colsb", [128, 8], F32, kind="ExternalInput").ap()
    NBF = 32 * NGROUPS + 128 + 128 + 32 + 32 + 2
    cbfd = nc.dram_tensor("cbf", [128, NBF], BF16, kind="ExternalInput").ap()
    cf32d = nc.dram_tensor("cf32", [128, npair + 8], F32, kind="ExternalInput").ap()
    outd = nc.dram_tensor("out", [D, nb], F32, kind="ExternalOutput").ap()

    clusters = _pack_clusters(sched)
    z1w = 512 * Z1_BANKS

    with tile.TileContext(nc) as tc:
        with (
            tc.tile_pool(name="const", bufs=1) as cp,
            tc.tile_pool(name="load", bufs=LP_BUFS) as lp,
            tc.tile_pool(name="kt", bufs=KT_BUFS) as ktp,
            tc.tile_pool(name="work", bufs=WP_BUFS) as wp,
            tc.tile_pool(name="h2p", bufs=H2_BUFS) as h2p,
            tc.tile_pool(name="outp", bufs=2) as op_,
            tc.tile_pool(name="ps1", bufs=PS1_BUFS, space="PSUM") as ps1,
            tc.tile_pool(name="ps2", bufs=PS2_BUFS, space="PSUM") as ps2,
        ):
            # ---- constants into SBUF: 3 packed DMAs issued from the PE queue
            # so the sync/scalar queues lead with the kT transposes
            NBF = 32 * NGROUPS + 128 + 128 + 32 + 32 + 2
            c_bf = cp.tile([128, NBF], BF16)
            nc.scalar.dma_start(out=c_bf[:], in_=cbfd)
            c_f32 = cp.tile([128, npair + 8], F32)
            nc.scalar.dma_start(out=c_f32[:], in_=cf32d)
            c_qw1 = cp.tile([1, 128 * npair], BF16)
            nc.scalar.dma_start(out=c_qw1[:], in_=qw1)
            o_ = 0
            c_mask = c_bf[:, o_ : o_ + 32 * NGROUPS]; o_ += 32 * NGROUPS
            c_wk = c_bf[:, o_ : o_ + 128]; o_ += 128
            c_wqk = c_bf[:, o_ : o_ + 128]; o_ += 128
            c_w2a = c_bf[:, o_ : o_ + 32]; o_ += 32
            c_w2na = c_bf[:, o_ : o_ + 32]; o_ += 32
            c_w34 = c_bf[:, o_ : o_ + 2]
            c_qp = c_f32[:, 0:npair]
            c_cols = c_f32[:, npair : npair + 8]
            c_ones = cp.tile([1, 128], BF16)
            nc.gpsimd.memset(c_ones[:], 1.0)

            gf_by_lb = {}
            kt_by_lg = {}
            po_tiles = {}
            done_in_blk = {}

            def ensure_block(lb):
                if lb in gf_by_lb:
                    return gf_by_lb[lb]
                TL1 = max(sched[i][0] for i in range(4 * lb, 4 * lb + 4))
                TL2 = max(sched[i][1] for i in range(4 * lb, 4 * lb + 4))
                boff = lb * ST * 64 * D
                gfa = lp.tile([128, 2, 64, 64], BF16, tag="gf")
                nc.gpsimd.dma_start(
                    out=gfa[0:TL1, 0, :, :],
                    in_=bass.AP(keyt.tensor, boff,
                                [[64 * D, TL1], [D, 64], [1, D]]),
                )
                if TL2 > 0:
                    nc.gpsimd.dma_start(
                        out=gfa[0:TL2, 1, :, :],
                        in_=bass.AP(keyt.tensor, boff + 128 * 64 * D,
                                    [[64 * D, TL2], [D, 64], [1, D]]),
                    )
                gf_by_lb[lb] = gfa
                return gfa

            def ensure_kt(lg):
                if lg in kt_by_lg:
                    return kt_by_lg[lg]
                gfa = ensure_block(lg // 2)
                T1lg = max(sched[2 * lg][0], sched[2 * lg + 1][0])
                T2lg = max(sched[2 * lg][1], sched[2 * lg + 1][1])
                tro = 32 * (lg % 2)
                eng1 = nc.sync
                eng2 = nc.sync
                ktf = ktp.tile([128, 16, 128], BF16, tag="ktf")
                eng1.dma_start(
                    out=ktf[:, :, 0:T1lg],
                    in_=gfa[0:T1lg, 0, tro : tro + 32, :].rearrange("p b d -> p (b d)"),
                    transpose=True,
                )
                ktq = None
                if T2lg > 0:
                    ktq = ktp.tile([128, 16, 80], BF16, tag="ktq")
                    eng2.dma_start(
                        out=ktq[:, :, 0:T2lg],
                        in_=gfa[0:T2lg, 1, tro : tro + 32, :].rearrange("p b d -> p (b d)"),
                        transpose=True,
                    )
                kt_by_lg[lg] = (ktf, ktq)
                return ktf, ktq

            pending_tail = []

            def emit_tail(t):
                (cl, ps2t) = t
                ng = len(cl["groups"])
                g0 = cl["groups"][0]
                prs = cl["pairs"]
                np_ = len(prs)
                p0 = prs[0]
                sc0 = cl["c2span"]
                # sg over all groups' score regions: [[48, ng], [1, 32]]
                sgt = wp.tile([128, ng, 32], BF16, tag="sg")
                sc_sc = ps2t[:, sc0 : sc0 + 48 * ng].rearrange(
                    "p (n c) -> p n c", n=ng)[:, :, 0:32]
                nc.scalar.activation(sgt[:], sc_sc, AF.Sigmoid)
                wtt = wp.tile([128, ng, 32], BF16, tag="wt")
                mk = c_mask[:, 32 * g0 : 32 * (g0 + ng)].rearrange(
                    "p (n c) -> p n c", n=ng)
                (nc.gpsimd if WT_POOL else nc.vector).tensor_tensor(
                    wtt[:], sgt[:], mk, ALU.mult)

                # pooling per group (po col r = 2*j + bh)
                for i, g in enumerate(cl["groups"]):
                    T1, T2 = sched[g]
                    lb = g // 4
                    gfa = gf_by_lb[lb]
                    ro = 16 * (g % 4)
                    scb = cl["scbase"][g]
                    po = ps2t[0:64, scb + 32 : scb + 48]
                    for j in prs:
                        for bh in range(2):
                            r = 2 * j + bh
                            nc.tensor.matmul(po[:, r : r + 1],
                                             gfa[0:T1, 0, ro + r, :],
                                             wtt[0:T1, i, 4 * j + bh : 4 * j + bh + 1],
                                             start=True, stop=(T2 == 0))
                            if T2 > 0:
                                nc.tensor.matmul(
                                    po[:, r : r + 1], gfa[0:T2, 1, ro + r, :],
                                    wtt[0:T2, i, 4 * j + 2 + bh : 4 * j + 3 + bh],
                                    start=False, stop=True)

                # po copy for the cluster (one op) + block flush
                ob = g0 // 4
                if ob not in po_tiles:
                    po_tiles[ob] = op_.tile([64, 64], F32, tag="posb", name="posb")
                    done_in_blk[ob] = 0
                po_sb = po_tiles[ob]
                if ng > 1:
                    po_in = ps2t[0:64, sc0 : sc0 + 48 * ng].rearrange(
                        "p (n c) -> p n c", n=ng)[:, :, 32:48]
                    nc.vector.tensor_copy(
                        po_sb[:, 16 * (g0 % 4) : 16 * (g0 % 4) + 16 * ng], po_in)
                else:
                    nc.vector.tensor_copy(
                        po_sb[:, 16 * (g0 % 4) + 2 * p0 : 16 * (g0 % 4) + 2 * p0 + 2 * np_],
                        ps2t[0:64, sc0 + 32 + 2 * p0 : sc0 + 32 + 2 * p0 + 2 * np_])
                done_in_blk[ob] += ng * np_ if ng > 1 else np_
                if done_in_blk[ob] == 32:
                    nc.sync.dma_start(
                        out=outd[:, 64 * ob : 64 * ob + 64], in_=po_tiles[ob][:]
                    )

            for ci, cl in enumerate(clusters):
                groups = cl["groups"]
                pairs = cl["pairs"]
                span = cl["span"]
                c2span = cl["c2span"]
                ng = len(groups)

                for g in groups:
                    ensure_block(g // 4)
                    ensure_kt(g // 2)

                # ---- per-pair q∘Wqk weights
                wqt = {}
                for g in groups:
                    wqv = wp.tile([128, 8, 128], BF16, tag="wq")
                    for j in pairs:
                        pj = 8 * g + j
                        eng = nc.gpsimd if j in WQ_POOL else nc.vector
                        eng.tensor_scalar(
                            wqv[:, j, :], c_wqk[:], c_qp[:, pj : pj + 1], None,
                            ALU.mult)
                    wqt[g] = wqv

                # ---- z1 super-tile: 3 accumulating matmuls per chunk
                z1 = ps1.tile([128, z1w], F32, tag="z1")
                if ci < PS1_BUFS and PSUM_INIT != "none":
                    if PSUM_INIT == "act":
                        nc.scalar.memzero(z1[:])
                    else:
                        nc.vector.memset(z1[:], 0.0)
                for g in groups:
                    T1, T2 = sched[g]
                    lg = g // 2
                    jo = 8 * (g % 2)
                    ktf, ktq = kt_by_lg[lg]
                    wq = wqt[g]
                    for j in pairs:
                        pj = 8 * g + j
                        qwj = c_qw1[0:1, 128 * pj : 128 * pj + 128]
                        o = cl["chunks"][(g, j, 0)]
                        nc.tensor.matmul(z1[:, o : o + T1], qwj, c_ones[0:1, 0:T1],
                                         start=True, stop=False)
                        nc.tensor.matmul(z1[:, o : o + T1], c_wk[:],
                                         ktf[:, jo + j, 0:T1],
                                         start=False, stop=False)
                        nc.tensor.matmul(z1[:, o : o + T1], wq[:, j, :],
                                         ktf[:, jo + j, 0:T1],
                                         start=False, stop=True)
                        if T2 > 0:
                            o2 = cl["chunks"][(g, j, 1)]
                            nc.tensor.matmul(z1[:, o2 : o2 + T2], qwj,
                                             c_ones[0:1, 0:T2], start=True, stop=False)
                            nc.tensor.matmul(z1[:, o2 : o2 + T2], c_wk[:],
                                             ktq[:, jo + j, 0:T2],
                                             start=False, stop=False)
                            nc.tensor.matmul(z1[:, o2 : o2 + T2], wq[:, j, :],
                                             ktq[:, jo + j, 0:T2],
                                             start=False, stop=True)

                # ---- dice1 over the whole cluster span (dead cols included)
                p1t = wp.tile([128, z1w], BF16, tag="p1")
                nc.scalar.activation(p1t[:, 0:span], z1[:, 0:span], AF.Sigmoid,
                                     bias=c_cols[:, 1:2], scale=c_cols[:, 0:1])
                if R2B_PAT:
                    r2b = R2B_PAT[ci % len(R2B_PAT)] == "1"
                else:
                    r2b = (ci * R2B_NUM) % R2B_DEN < R2B_NUM

                ps2t = ps2.tile([128, 512], F32, tag="z2")
                if ci < PS2_BUFS and PSUM_INIT != "none":
                    if PSUM_INIT == "act":
                        nc.scalar.memzero(ps2t[:])
                    else:
                        nc.vector.memset(ps2t[:], 0.0)
                if r2b:
                    x1c = wp.tile([128, z1w], BF16, tag="x1c")
                    nc.scalar.copy(x1c[:, 0:span], z1[:, 0:span])
                    h1t = wp.tile([128, z1w], BF16, tag="u1")
                    nc.vector.tensor_tensor(h1t[:, 0:span], x1c[:, 0:span],
                                            p1t[:, 0:span], ALU.mult)
                else:
                    h1t = wp.tile([128, z1w], BF16, tag="h1")
                    nc.vector.scalar_tensor_tensor(
                        h1t[:, 0:span], p1t[:, 0:span], c_cols[:, 2:3],
                        z1[:, 0:span], ALU.add, ALU.mult)

                # ---- z2 matmuls into the cluster ps2 tile
                for g in groups:
                    T1, T2 = sched[g]
                    Teff = T1 + T2
                    zb = cl["z2base"][g]
                    for j in pairs:
                        b = j // 2
                        co = zb + (j % 2) * Teff
                        for part, T, cod in ((0, T1, 0), (1, T2, T1)):
                            if T == 0:
                                continue
                            o = cl["chunks"][(g, j, part)]
                            if r2b:
                                nc.tensor.matmul(
                                    ps2t[32 * b : 32 * b + 32, co + cod : co + cod + T],
                                    c_w2a[:], x1c[:, o : o + T],
                                    start=True, stop=False,
                                    tile_position=(0, 32 * b))
                                nc.tensor.matmul(
                                    ps2t[32 * b : 32 * b + 32, co + cod : co + cod + T],
                                    c_w2na[:], h1t[:, o : o + T],
                                    start=False, stop=True,
                                    tile_position=(0, 32 * b))
                            else:
                                nc.tensor.matmul(
                                    ps2t[32 * b : 32 * b + 32, co + cod : co + cod + T],
                                    c_w2na[:], h1t[:, o : o + T],
                                    start=True, stop=True,
                                    tile_position=(0, 32 * b))

                # ---- dice2 for the cluster
                p2t = wp.tile([128, 512], BF16, tag="p2")
                nc.scalar.activation(p2t[:, 0:c2span], ps2t[:, 0:c2span], AF.Sigmoid,
                                     bias=c_cols[:, 5:6], scale=c_cols[:, 4:5])
                h2t = h2p.tile([128, 512], BF16, tag="h2")
                if H2ACT_PAT and H2ACT_PAT[ci % len(H2ACT_PAT)] == "1":
                    x2c = wp.tile([128, 512], BF16, tag="x2c")
                    nc.scalar.copy(x2c[:, 0:c2span], ps2t[:, 0:c2span])
                    p2m = wp.tile([128, 512], BF16, tag="p2m")
                    nc.vector.tensor_scalar(p2m[:, 0:c2span], p2t[:, 0:c2span],
                                            c_cols[:, 6:7], None, ALU.add)
                    nc.vector.tensor_tensor(h2t[:, 0:c2span], x2c[:, 0:c2span],
                                            p2m[:, 0:c2span], ALU.mult)
                else:
                    nc.vector.scalar_tensor_tensor(
                        h2t[:, 0:c2span], p2t[:, 0:c2span], c_cols[:, 6:7],
                        ps2t[:, 0:c2span], ALU.add, ALU.mult)

                # ---- scores into per-group sc regions of ps2t
                for g in groups:
                    T1, T2 = sched[g]
                    Teff = T1 + T2
                    zb = cl["z2base"][g]
                    scb = cl["scbase"][g]
                    for j in pairs:
                        b = j // 2
                        co = zb + (j % 2) * Teff
                        nc.tensor.matmul(ps2t[0:T1, scb + 4 * j : scb + 4 * j + 2],
                                         h2t[32 * b : 32 * b + 32, co : co + T1],
                                         c_w34[32 * b : 32 * b + 32, :],
                                         start=True, stop=True,
                                         tile_position=(32 * b, 0))
                        if T2 > 0:
                            nc.tensor.matmul(
                                ps2t[0:T2, scb + 4 * j + 2 : scb + 4 * j + 4],
                                h2t[32 * b : 32 * b + 32, co + T1 : co + Teff],
                                c_w34[32 * b : 32 * b + 32, :],
                                start=True, stop=True,
                                tile_position=(32 * b, 0))

                # ---- deferred tail emission (software pipelining)
                this_tail = (cl, ps2t)
                if TAIL_DEFER:
                    if pending_tail:
                        emit_tail(pending_tail.pop())
                    pending_tail.append(this_tail)
                else:
                    emit_tail(this_tail)
            if pending_tail:
                emit_tail(pending_tail.pop())
    nc.compile()
    return nc


def _blk(a):
    m = np.zeros((128, 2 * a.shape[1]), np.float32)
    m[0:64, 0 : a.shape[1]] = a
    m[64:128, a.shape[1] :] = a
    return m


def _prep_consts(W1, alpha1, mean1, var1, W2, alpha2, mean2, var2, W3):
    inv1 = 1.0 / np.sqrt(var1 + EPS)
    inv2 = 1.0 / np.sqrt(var2 + EPS)
    Wq = W1[0:64] + W1[128:192]
    Wk = W1[64:128] - W1[128:192]
    Wqk = W1[192:256]

    wk2 = _blk(Wk).astype(bf)
    wqk2 = _blk(Wqk).astype(bf)
    w2a = _blk(np.diag(alpha1) @ W2).astype(bf)
    w2na = _blk(np.diag(1.0 - alpha1) @ W2).astype(bf)
    w34p = np.zeros((32, 2), np.float32)
    w34p[0:16, 0] = W3[:, 0] * (1.0 - alpha2)
    w34p[16:32, 1] = W3[:, 0] * (1.0 - alpha2)
    w34 = np.tile(w34p, (4, 1)).astype(bf)
    colsb = np.zeros((128, 8), np.float32)
    colsb[:, 0] = np.tile(inv1, 2)
    colsb[:, 1] = np.tile(-mean1 * inv1, 2)
    colsb[:, 2] = np.tile(alpha1 / (1.0 - alpha1), 2)
    colsb[:, 3] = np.tile(alpha1, 2)
    colsb[:, 4] = np.tile(inv2, 8)
    colsb[:, 5] = np.tile(-mean2 * inv2, 8)
    colsb[:, 6] = np.tile(alpha2 / (1.0 - alpha2), 8)
    colsb[:, 7] = np.tile(alpha2, 8)
    return Wq, wk2, wqk2, w2a, w2na, w34, colsb


def kernel(query_emb, key_emb, seq_length, W1, alpha1, mean1, var1,
           W2, alpha2, mean2, var2, W3):
    (Wq, wk2, wqk2, w2a, w2na, w34, colsb) = _prep_consts(
        np.asarray(W1, np.float32), np.asarray(alpha1, np.float32),
        np.asarray(mean1, np.float32), np.asarray(var1, np.float32),
        np.asarray(W2, np.float32), np.asarray(alpha2, np.float32),
        np.asarray(mean2, np.float32), np.asarray(var2, np.float32),
        np.asarray(W3, np.float32))
    q = np.asarray(query_emb, np.float32)
    k = np.asarray(key_emb, np.float32)
    sl = np.asarray(seq_length).reshape(-1).astype(np.int64)

    qW = (q @ Wq).astype(np.float32)  # [B, 64]

    order = np.argsort(sl, kind="stable")
    shards = [order[c::NCORES] for c in range(NCORES)]

    sched = []
    for g in range(NGROUPS):
        mx = max(int(sl[shards[c][16 * g : 16 * g + 16]].max()) for c in range(NCORES))
        sched.append((min(128, _ceil16(mx)), _ceil16(max(0, mx - 128))))
    sched = tuple(sched)

    if sched not in _CACHE:
        _CACHE[sched] = _build(sched)
    nc = _CACHE[sched]
    npair = BLOC // 2

    t_full = np.arange(128)[:, None]
    t_part = np.arange(128)[:, None] + 128

    in_maps = []
    for c in range(NCORES):
        rows = shards[c]
        slc = sl[rows]
        qs = q[rows]          # [256, 64]
        qWs = qW[rows]        # [256, 64]

        qp_t = np.zeros((128, npair), np.float32)
        qp_t[0:64] = qs[0::2].T
        qp_t[64:128] = qs[1::2].T

        qw1_t = np.zeros((1, 128 * npair), np.float32)
        qw1_r = qw1_t.reshape(npair, 2, 64)
        qw1_r[:, 0, :] = qWs[0::2]
        qw1_r[:, 1, :] = qWs[1::2]

        mk = np.zeros((128, 32 * NGROUPS), np.float32)
        for g in range(NGROUPS):
            sg_ = slc[16 * g : 16 * g + 16]
            full = (t_full < sg_[None, :]).astype(np.float32)   # [128, 16]
            part = (t_part < sg_[None, :]).astype(np.float32)
            mk[:, 32 * g + 0 : 32 * g + 32 : 4] = full[:, 0::2]
            mk[:, 32 * g + 1 : 32 * g + 32 : 4] = full[:, 1::2]
            mk[:, 32 * g + 2 : 32 * g + 32 : 4] = part[:, 0::2]
            mk[:, 32 * g + 3 : 32 * g + 32 : 4] = part[:, 1::2]

        ks_ = k[rows]  # [256, 200, 64]
        keyt_h = np.zeros((4, 256, 64, D), np.float32)
        for b_ in range(4):
            keyt_h[b_, 0:S] = ks_[64 * b_ : 64 * b_ + 64].transpose(1, 0, 2)
        cbf_h = np.concatenate(
            [mk, wk2, wqk2, w2a, w2na, w34], axis=1).astype(bf)
        cf32_h = np.concatenate([qp_t, colsb], axis=1).astype(np.float32)
        in_maps.append({
            "keyt": keyt_h.reshape(4 * 256 * 64, D),
            "qp": qp_t,
            "qw1": qw1_t.astype(bf),
            "maskd": mk.astype(bf),
            "wk2": wk2, "wqk2": wqk2, "w2a": w2a, "w2na": w2na,
            "w34": w34, "colsb": colsb,
            "cbf": cbf_h, "cf32": cf32_h,
        })

    res = run_bass_kernel_spmd(nc, in_maps, list(range(NCORES)), trace=TRACE)
    global LAST_RESULT
    LAST_RESULT = res

    out_full = np.zeros((B, D), np.float32)
    for c in range(NCORES):
        out_full[shards[c]] = np.asarray(res.results[c]["out"], np.float32).T
    return out_full


# revision 57
# speedup vs baseline: 1.1514x; 1.1514x over previous
"""AttentionSequencePoolingLayer Trainium2 kernel (8-core data parallel), v3.

B=2048, S=200, D=64, H1=64, H2=16. Batch sharded 256/core.

Strategy:
- Rows are globally sorted by seq_length and dealt round-robin to the 8 cores,
  so every core sees the same length profile. Within a core, rows are grouped
  16 at a time; group g only processes T_g = roundup(max seq_length, 16)
  tokens (ΣT ≈ 0.57 × S). One program (compiled per schedule) serves all cores.
- k is re-laid token-major per 64-row block on the host (8KB DMA descriptors,
  zero-padded to 256 tokens); cast-load bf16 (tokens on partitions), xbar-transpose
  to kT [(bhat,d), tok] pair tiles.
- Groups are CLUSTERED into 3-PSUM-bank z1 super-tiles (first-fit chunk packing,
  dead columns allowed and simply processed as garbage): one sigmoid + one
  fused dice op per cluster instead of per ~64-token unit.
- z1 = x1 = att@W1 in PSUM via 3 accumulating matmuls per pair chunk:
  qW ⊗ ones (K=1 rank-1), Wk^T kT, (q∘Wqk)^T kT.
- dice via scalar_tensor_tensor with (1-alpha) folded into the next weights:
  dice(x) = (1-a)*x*(p + a/(1-a)); w2na = (1-a1)W2, w34 = (1-a2)W3.
  Route r1: DVE stt reads z1 PSUM directly. Route r2b (rare): ACT copy
  x1->SBUF, DVE 2x tensor_tensor, z2 = w2a^T x1 + w2na^T u1.
- z2 + scores + pooling PSUM share one bank per cluster (z2 region | per-group
  48-col score/pool regions); dice2 = one sigmoid + one stt per cluster.
- scores via tiny N=2 matmuls into token-major PSUM; σ+mask; pooling as
  out[d,1] = k_nat^T w with N=1 matmuls (k stationary); d-major DRAM output,
  transposed on host at gather.
- Emission is software-pipelined: cluster c's tail (sg/wt/pooling/po-copy) is
  emitted after cluster c+1's head, so in-order engine queues don't
  head-of-line block the next cluster's big ops.
"""
import numpy as np
import ml_dtypes

import concourse.bacc as bacc
import concourse.tile as tile
import concourse.mybir as mybir
import concourse.bass as bass
from concourse.bass_utils import run_bass_kernel_spmd

B, S, D = 2048, 200, 64
H1, H2 = 64, 16
EPS = 1e-9
NCORES = 8
BLOC = B // NCORES          # 256 rows per core
NGROUPS = BLOC // 16        # 16

F32 = mybir.dt.float32
BF16 = mybir.dt.bfloat16
AF = mybir.ActivationFunctionType
ALU = mybir.AluOpType
bf = ml_dtypes.bfloat16

# fraction of clusters on route-2b (ACT-heavy) vs route-1 (DVE-heavy): num/den
R2B_NUM, R2B_DEN = 0, 8
R2B_PAT = ""
LP_BUFS, KT_BUFS, WP_BUFS, H2_BUFS = 3, 4, 10, 2
PS1_BUFS, PS2_BUFS = 2, 4
Z1_BANKS = 2            # banks per z1 super-tile (512 f32 cols each)
TAIL_DEFER = True
WT_POOL = False
CSPLIT = False
PSUM_INIT = "none"       # 'none' | 'act' | 'dve' one-time PSUM buf init
TENG = "sync"            # 'sync' | 'alt' transpose DMA queue assignment
CONST_ENG = "sync"     # 'scalar' | 'gpsimd' packed-const DMA queue
HOIST01 = False          # emit block0 load + first transposes before consts
LOAD_ENG = 'gpsimd'      # 'gpsimd' | 'sync' keyt block-load queue
WQH_GE = 99              # groups >= this use host-precomputed q*Wqk (DMA'd)
WQH_ENG = 'gpsimd'       # queue for wqh loads
SPLIT_GE = 99           # groups >= this index are force-split into pair-halves
WQ_POOL = ()            # pair indices j whose q*Wqk build runs on Pool
H2ACT_PAT = ""          # per-cluster '1' = dice2 via ACT copy + 2x DVE mult
LOAD_BLOCKS = ((0, 4), (4, 4), (8, 4), (12, 4))  # (start_group, ngroups)

_CACHE = {}
TRACE = False
LAST_RESULT = None


def _ceil16(x):
    return (int(x) + 15) // 16 * 16


def _pack_clusters(sched):
    """Partition groups into clusters; pack each pair-chunk (T1/T2 separately)
    into 512-col banks of a Z1_BANKS-bank z1 super-tile, never crossing a bank
    boundary. Returns list of clusters:
      {groups: [g..], span: int, chunks: {(g, j, part): col}, z2base: {g: col},
       c2span: int, scbase: {g: col}}
    Clusters never cross 4-group load-block boundaries. A group whose chunks
    do not fit in an empty tile is split into two pair-halves.
    """
    clusters = []

    def new_cluster():
        return {"groups": [], "banks": [0] * Z1_BANKS, "chunks": {},
                "span": 0, "z2base": {}, "c2span": 0, "scbase": {}}

    def try_place(cl, g, pairs, record=True):
        T1, T2 = sched[g]
        banks = list(cl["banks"])
        place = {}
        for part, T in ((0, T1), (1, T2)):
            if T == 0:
                continue
            for j in pairs:
                for b in range(Z1_BANKS):
                    if banks[b] + T <= 512:
                        place[(g, j, part)] = 512 * b + banks[b]
                        banks[b] += T
                        break
                else:
                    return False
        if record:
            cl["banks"] = banks
            cl["chunks"].update(place)
            cl["span"] = max(512 * b + banks[b] for b in range(Z1_BANKS)
                             if banks[b] > 0)
        return True

    g = 0
    while g < NGROUPS:
        T1, T2 = sched[g]
        Teff = T1 + T2
        cl = clusters[-1] if clusters else None
        # C2/sc budget: z2 region + 48 cols per group must fit one 512-col bank
        def fits_c2(cl_, add_groups):
            ng = len(cl_["groups"]) + add_groups if cl_ else add_groups
            c2 = (cl_["c2span"] if cl_ else 0) + 2 * Teff * add_groups
            return c2 + 48 * ng <= 512
        if (cl is not None and g % 4 != 0 and len(cl["groups"]) > 0
                and fits_c2(cl, 1) and try_place(cl, g, range(8), record=False)
                and cl["groups"][0] // 4 == g // 4):
            try_place(cl, g, range(8))
            cl["z2base"][g] = cl["c2span"]
            cl["c2span"] += 2 * Teff
            cl["groups"].append(g)
        else:
            cl = new_cluster()
            clusters.append(cl)
            if g < SPLIT_GE and try_place(cl, g, range(8)):
                cl["z2base"][g] = 0
                cl["c2span"] = 2 * Teff
                cl["groups"].append(g)
            else:
                # split group into two half-pair clusters
                clusters.pop()
                for half in (range(0, 4), range(4, 8)):
                    cl = new_cluster()
                    clusters.append(cl)
                    assert try_place(cl, g, half)
                    cl["z2base"][g] = 0
                    cl["c2span"] = 2 * Teff  # 4 pairs * 2 slots... see below
                    cl["groups"].append(g)
                    cl["pairs"] = tuple(half)
        g += 1

    for cl in clusters:
        ng = len(cl["groups"])
        for i, gg in enumerate(cl["groups"]):
            cl["scbase"][gg] = cl["c2span"] + 48 * i
        cl.setdefault("pairs", tuple(range(8)))
    return clusters


def _build(sched):
    nc = bacc.Bacc("TRN2", target_bir_lowering=False, debug=False, num_devices=NCORES,
                   dynamic_dma_scratch_size=16384)
    nb = BLOC
    npair = nb // 2  # 128

    ST = 256  # padded tokens per load block (two 128-token chunks)
    keyt = nc.dram_tensor("keyt", [4 * ST * 64, D], BF16, kind="ExternalInput").ap()
    qp = nc.dram_tensor("qp", [128, npair], F32, kind="ExternalInput").ap()
    qw1 = nc.dram_tensor("qw1", [1, 128 * npair], BF16, kind="ExternalInput").ap()
    maskd = nc.dram_tensor("maskd", [128, 32 * NGROUPS], BF16, kind="ExternalInput").ap()
    wk2 = nc.dram_tensor("wk2", [128, 128], BF16, kind="ExternalInput").ap()
    wqk2 = nc.dram_tensor("wqk2", [128, 128], BF16, kind="ExternalInput").ap()
    w2a = nc.dram_tensor("w2a", [128, 32], BF16, kind="ExternalInput").ap()
    w2na = nc.dram_tensor("w2na", [128, 32], BF16, kind="ExternalInput").ap()
    w34 = nc.dram_tensor("w34", [128, 2], BF16, kind="ExternalInput").ap()
    colsb = nc.dram_tensor("colsb", [128, 8], F32, kind="ExternalInput").ap()
    NBF = 32 * NGROUPS + 128 + 128 + 32 + 32 + 2
    cbfd = nc.dram_tensor("cbf", [128, NBF], BF16, kind="ExternalInput").ap()
    cf32d = nc.dram_tensor("cf32", [128, npair + 8], F32, kind="ExternalInput").ap()
    n_wqh = max(1, (NGROUPS - WQH_GE) * 8)
    wqh = nc.dram_tensor("wqh", [128, n_wqh * 128], BF16, kind="ExternalInput").ap()
    outd = nc.dram_tensor("out", [D, nb], F32, kind="ExternalOutput").ap()

    clusters = _pack_clusters(sched)
    z1w = 512 * Z1_BANKS

    with tile.TileContext(nc) as tc:
        with (
            tc.tile_pool(name="const", bufs=1) as cp,
            tc.tile_pool(name="load", bufs=LP_BUFS) as lp,
            tc.tile_pool(name="kt", bufs=KT_BUFS) as ktp,
            tc.tile_pool(name="work", bufs=WP_BUFS) as wp,
            tc.tile_pool(name="h2p", bufs=H2_BUFS) as h2p,
            tc.tile_pool(name="outp", bufs=2) as op_,
            tc.tile_pool(name="ps1", bufs=PS1_BUFS, space="PSUM") as ps1,
            tc.tile_pool(name="ps2", bufs=PS2_BUFS, space="PSUM") as ps2,
        ):
            gf_by_lb = {}
            kt_by_lg = {}
            po_tiles = {}
            done_in_blk = {}

            def ensure_block(lb):
                if lb in gf_by_lb:
                    return gf_by_lb[lb]
                TL1 = max(sched[i][0] for i in range(4 * lb, 4 * lb + 4))
                TL2 = max(sched[i][1] for i in range(4 * lb, 4 * lb + 4))
                boff = lb * ST * 64 * D
                gfa = lp.tile([128, 2, 64, 64], BF16, tag="gf")
                lde = nc.sync if LOAD_ENG == "sync" else nc.gpsimd
                lde.dma_start(
                    out=gfa[0:TL1, 0, :, :],
                    in_=bass.AP(keyt.tensor, boff,
                                [[64 * D, TL1], [D, 64], [1, D]]),
                )
                if TL2 > 0:
                    lde.dma_start(
                        out=gfa[0:TL2, 1, :, :],
                        in_=bass.AP(keyt.tensor, boff + 128 * 64 * D,
                                    [[64 * D, TL2], [D, 64], [1, D]]),
                    )
                gf_by_lb[lb] = gfa
                return gfa

            def ensure_kt(lg):
                if lg in kt_by_lg:
                    return kt_by_lg[lg]
                gfa = ensure_block(lg // 2)
                T1lg = max(sched[2 * lg][0], sched[2 * lg + 1][0])
                T2lg = max(sched[2 * lg][1], sched[2 * lg + 1][1])
                tro = 32 * (lg % 2)
                if TENG == "sync":
                    eng1 = eng2 = nc.sync
                else:
                    eng1 = nc.sync if lg % 2 == 0 else nc.scalar
                    eng2 = nc.scalar if lg % 2 == 0 else nc.sync
                ktf = ktp.tile([128, 16, 128], BF16, tag="ktf")
                eng1.dma_start(
                    out=ktf[:, :, 0:T1lg],
                    in_=gfa[0:T1lg, 0, tro : tro + 32, :].rearrange("p b d -> p (b d)"),
                    transpose=True,
                )
                ktq = None
                if T2lg > 0:
                    ktq = ktp.tile([128, 16, 80], BF16, tag="ktq")
                    eng2.dma_start(
                        out=ktq[:, :, 0:T2lg],
                        in_=gfa[0:T2lg, 1, tro : tro + 32, :].rearrange("p b d -> p (b d)"),
                        transpose=True,
                    )
                kt_by_lg[lg] = (ktf, ktq)
                return ktf, ktq

            if HOIST01:
                ensure_block(0)
                ensure_kt(0)
                ensure_kt(1)
            cde = {'gpsimd': nc.gpsimd, 'scalar': nc.scalar, 'sync': nc.sync}[CONST_ENG]
            # ---- constants into SBUF: 3 packed DMAs issued from the PE queue
            # so the sync/scalar queues lead with the kT transposes
            NBF = 32 * NGROUPS + 128 + 128 + 32 + 32 + 2
            c_bf = cp.tile([128, NBF], BF16)
            cde.dma_start(out=c_bf[:], in_=cbfd)
            c_f32 = cp.tile([128, npair + 8], F32)
            cde.dma_start(out=c_f32[:], in_=cf32d)
            c_qw1 = cp.tile([1, 128 * npair], BF16)
            cde.dma_start(out=c_qw1[:], in_=qw1)
            o_ = 0
            c_mask = c_bf[:, o_ : o_ + 32 * NGROUPS]; o_ += 32 * NGROUPS
            c_wk = c_bf[:, o_ : o_ + 128]; o_ += 128
            c_wqk = c_bf[:, o_ : o_ + 128]; o_ += 128
            c_w2a = c_bf[:, o_ : o_ + 32]; o_ += 32
            c_w2na = c_bf[:, o_ : o_ + 32]; o_ += 32
            c_w34 = c_bf[:, o_ : o_ + 2]
            c_qp = c_f32[:, 0:npair]
            c_cols = c_f32[:, npair : npair + 8]
            c_ones = cp.tile([1, 128], BF16)
            nc.gpsimd.memset(c_ones[:], 1.0)


            pending_tail = []

            def emit_tail(t):
                (cl, ps2t) = t
                ng = len(cl["groups"])
                g0 = cl["groups"][0]
                prs = cl["pairs"]
                np_ = len(prs)
                p0 = prs[0]
                sc0 = cl["c2span"]
                # sg over all groups' score regions: [[48, ng], [1, 32]]
                sgt = wp.tile([128, ng, 32], BF16, tag="sg")
                sc_sc = ps2t[:, sc0 : sc0 + 48 * ng].rearrange(
                    "p (n c) -> p n c", n=ng)[:, :, 0:32]
                nc.scalar.activation(sgt[:], sc_sc, AF.Sigmoid)
                wtt = wp.tile([128, ng, 32], BF16, tag="wt")
                mk = c_mask[:, 32 * g0 : 32 * (g0 + ng)].rearrange(
                    "p (n c) -> p n c", n=ng)
                (nc.gpsimd if WT_POOL else nc.vector).tensor_tensor(
                    wtt[:], sgt[:], mk, ALU.mult)

                # pooling per group (po col r = 2*j + bh)
                for i, g in enumerate(cl["groups"]):
                    T1, T2 = sched[g]
                    lb = g // 4
                    gfa = gf_by_lb[lb]
                    ro = 16 * (g % 4)
                    scb = cl["scbase"][g]
                    po = ps2t[0:64, scb + 32 : scb + 48]
                    for j in prs:
                        for bh in range(2):
                            r = 2 * j + bh
                            nc.tensor.matmul(po[:, r : r + 1],
                                             gfa[0:T1, 0, ro + r, :],
                                             wtt[0:T1, i, 4 * j + bh : 4 * j + bh + 1],
                                             start=True, stop=(T2 == 0))
                            if T2 > 0:
                                nc.tensor.matmul(
                                    po[:, r : r + 1], gfa[0:T2, 1, ro + r, :],
                                    wtt[0:T2, i, 4 * j + 2 + bh : 4 * j + 3 + bh],
                                    start=False, stop=True)

                # po copy for the cluster (one op) + block flush
                ob = g0 // 4
                if ob not in po_tiles:
                    po_tiles[ob] = op_.tile([64, 64], F32, tag="posb", name="posb")
                    done_in_blk[ob] = 0
                po_sb = po_tiles[ob]
                if ng > 1:
                    po_in = ps2t[0:64, sc0 : sc0 + 48 * ng].rearrange(
                        "p (n c) -> p n c", n=ng)[:, :, 32:48]
                    nc.vector.tensor_copy(
                        po_sb[:, 16 * (g0 % 4) : 16 * (g0 % 4) + 16 * ng], po_in)
                else:
                    nc.vector.tensor_copy(
                        po_sb[:, 16 * (g0 % 4) + 2 * p0 : 16 * (g0 % 4) + 2 * p0 + 2 * np_],
                        ps2t[0:64, sc0 + 32 + 2 * p0 : sc0 + 32 + 2 * p0 + 2 * np_])
                done_in_blk[ob] += ng * np_ if ng > 1 else np_
                if done_in_blk[ob] == 32:
                    nc.sync.dma_start(
                        out=outd[:, 64 * ob : 64 * ob + 64], in_=po_tiles[ob][:]
                    )

            for ci, cl in enumerate(clusters):
                groups = cl["groups"]
                pairs = cl["pairs"]
                span = cl["span"]
                c2span = cl["c2span"]
                ng = len(groups)

                for g in groups:
                    ensure_block(g // 4)
                    ensure_kt(g // 2)

                # ---- per-pair q∘Wqk weights
                wqt = {}
                for g in groups:
                    wqv = wp.tile([128, 8, 128], BF16, tag="wq")
                    if g >= WQH_GE:
                        off = (g - WQH_GE) * 8 * 128
                        weng = {"gpsimd": nc.gpsimd, "sync": nc.sync,
                                "scalar": nc.scalar}[WQH_ENG]
                        weng.dma_start(
                            out=wqv[:],
                            in_=wqh[:, off : off + 8 * 128])
                    else:
                        for j in pairs:
                            pj = 8 * g + j
                            eng = nc.gpsimd if j in WQ_POOL else nc.vector
                            eng.tensor_scalar(
                                wqv[:, j, :], c_wqk[:], c_qp[:, pj : pj + 1], None,
                                ALU.mult)
                    wqt[g] = wqv

                # ---- z1 super-tile: 3 accumulating matmuls per chunk
                z1 = ps1.tile([128, z1w], F32, tag="z1")
                if ci < PS1_BUFS and PSUM_INIT != "none":
                    if PSUM_INIT == "act":
                        nc.scalar.memzero(z1[:])
                    else:
                        nc.vector.memset(z1[:], 0.0)
                for g in groups:
                    T1, T2 = sched[g]
                    lg = g // 2
                    jo = 8 * (g % 2)
                    ktf, ktq = kt_by_lg[lg]
                    wq = wqt[g]
                    for j in pairs:
                        pj = 8 * g + j
                        qwj = c_qw1[0:1, 128 * pj : 128 * pj + 128]
                        o = cl["chunks"][(g, j, 0)]
                        nc.tensor.matmul(z1[:, o : o + T1], qwj, c_ones[0:1, 0:T1],
                                         start=True, stop=False)
                        nc.tensor.matmul(z1[:, o : o + T1], c_wk[:],
                                         ktf[:, jo + j, 0:T1],
                                         start=False, stop=False)
                        nc.tensor.matmul(z1[:, o : o + T1], wq[:, j, :],
                                         ktf[:, jo + j, 0:T1],
                                         start=False, stop=True)
                        if T2 > 0:
                            o2 = cl["chunks"][(g, j, 1)]
                            nc.tensor.matmul(z1[:, o2 : o2 + T2], qwj,
                                             c_ones[0:1, 0:T2], start=True, stop=False)
                            nc.tensor.matmul(z1[:, o2 : o2 + T2], c_wk[:],
                                             ktq[:, jo + j, 0:T2],
                                             start=False, stop=False)
                            nc.tensor.matmul(z1[:, o2 : o2 + T2], wq[:, j, :],
                                             ktq[:, jo + j, 0:T2],
                                             start=False, stop=True)

                # ---- dice1 over the whole cluster span (dead cols included)
                p1t = wp.tile([128, z1w], BF16, tag="p1")
                nc.scalar.activation(p1t[:, 0:span], z1[:, 0:span], AF.Sigmoid,
                                     bias=c_cols[:, 1:2], scale=c_cols[:, 0:1])
                if R2B_PAT:
                    r2b = R2B_PAT[ci % len(R2B_PAT)] == "1"
                else:
                    r2b = (ci * R2B_NUM) % R2B_DEN < R2B_NUM

                ps2t = ps2.tile([128, 512], F32, tag="z2")
                if ci < PS2_BUFS and PSUM_INIT != "none":
                    if PSUM_INIT == "act":
                        nc.scalar.memzero(ps2t[:])
                    else:
                        nc.vector.memset(ps2t[:], 0.0)
                if r2b:
                    x1c = wp.tile([128, z1w], BF16, tag="x1c")
                    nc.scalar.copy(x1c[:, 0:span], z1[:, 0:span])
                    h1t = wp.tile([128, z1w], BF16, tag="u1")
                    nc.vector.tensor_tensor(h1t[:, 0:span], x1c[:, 0:span],
                                            p1t[:, 0:span], ALU.mult)
                else:
                    h1t = wp.tile([128, z1w], BF16, tag="h1")
                    nc.vector.scalar_tensor_tensor(
                        h1t[:, 0:span], p1t[:, 0:span], c_cols[:, 2:3],
                        z1[:, 0:span], ALU.add, ALU.mult)

                # ---- z2 matmuls into the cluster ps2 tile
                for g in groups:
                    T1, T2 = sched[g]
                    Teff = T1 + T2
                    zb = cl["z2base"][g]
                    for j in pairs:
                        b = j // 2
                        co = zb + (j % 2) * Teff
                        for part, T, cod in ((0, T1, 0), (1, T2, T1)):
                            if T == 0:
                                continue
                            o = cl["chunks"][(g, j, part)]
                            if r2b:
                                nc.tensor.matmul(
                                    ps2t[32 * b : 32 * b + 32, co + cod : co + cod + T],
                                    c_w2a[:], x1c[:, o : o + T],
                                    start=True, stop=False,
                                    tile_position=(0, 32 * b))
                                nc.tensor.matmul(
                                    ps2t[32 * b : 32 * b + 32, co + cod : co + cod + T],
                                    c_w2na[:], h1t[:, o : o + T],
                                    start=False, stop=True,
                                    tile_position=(0, 32 * b))
                            else:
                                nc.tensor.matmul(
                                    ps2t[32 * b : 32 * b + 32, co + cod : co + cod + T],
                                    c_w2na[:], h1t[:, o : o + T],
                                    start=True, stop=True,
                                    tile_position=(0, 32 * b))

                # ---- dice2 for the cluster
                p2t = wp.tile([128, 512], BF16, tag="p2")
                nc.scalar.activation(p2t[:, 0:c2span], ps2t[:, 0:c2span], AF.Sigmoid,
                                     bias=c_cols[:, 5:6], scale=c_cols[:, 4:5])
                h2t = h2p.tile([128, 512], BF16, tag="h2")
                if H2ACT_PAT and H2ACT_PAT[ci % len(H2ACT_PAT)] == "1":
                    x2c = wp.tile([128, 512], BF16, tag="x2c")
                    nc.scalar.copy(x2c[:, 0:c2span], ps2t[:, 0:c2span])
                    p2m = wp.tile([128, 512], BF16, tag="p2m")
                    nc.vector.tensor_scalar(p2m[:, 0:c2span], p2t[:, 0:c2span],
                                            c_cols[:, 6:7], None, ALU.add)
                    nc.vector.tensor_tensor(h2t[:, 0:c2span], x2c[:, 0:c2span],
                                            p2m[:, 0:c2span], ALU.mult)
                else:
                    nc.vector.scalar_tensor_tensor(
                        h2t[:, 0:c2span], p2t[:, 0:c2span], c_cols[:, 6:7],
                        ps2t[:, 0:c2span], ALU.add, ALU.mult)

                # ---- scores into per-group sc regions of ps2t
                for g in groups:
                    T1, T2 = sched[g]
                    Teff = T1 + T2
                    zb = cl["z2base"][g]
                    scb = cl["scbase"][g]
                    for j in pairs:
                        b = j // 2
                        co = zb + (j % 2) * Teff
                        nc.tensor.matmul(ps2t[0:T1, scb + 4 * j : scb + 4 * j + 2],
                                         h2t[32 * b : 32 * b + 32, co : co + T1],
                                         c_w34[32 * b : 32 * b + 32, :],
                                         start=True, stop=True,
                                         tile_position=(32 * b, 0))
                        if T2 > 0:
                            nc.tensor.matmul(
                                ps2t[0:T2, scb + 4 * j + 2 : scb + 4 * j + 4],
                                h2t[32 * b : 32 * b + 32, co + T1 : co + Teff],
                                c_w34[32 * b : 32 * b + 32, :],
                                start=True, stop=True,
                                tile_position=(32 * b, 0))

                # ---- deferred tail emission (software pipelining)
                this_tail = (cl, ps2t)
                if TAIL_DEFER:
                    if pending_tail:
                        emit_tail(pending_tail.pop())
                    pending_tail.append(this_tail)
                else:
                    emit_tail(this_tail)
            if pending_tail:
                emit_tail(pending_tail.pop())
    nc.compile()
    return nc


def _blk(a):
    m = np.zeros((128, 2 * a.shape[1]), np.float32)
    m[0:64, 0 : a.shape[1]] = a
    m[64:128, a.shape[1] :] = a
    return m


def _prep_consts(W1, alpha1, mean1, var1, W2, alpha2, mean2, var2, W3):
    inv1 = 1.0 / np.sqrt(var1 + EPS)
    inv2 = 1.0 / np.sqrt(var2 + EPS)
    Wq = W1[0:64] + W1[128:192]
    Wk = W1[64:128] - W1[128:192]
    Wqk = W1[192:256]

    wk2 = _blk(Wk).astype(bf)
    wqk2 = _blk(Wqk).astype(bf)
    w2a = _blk(np.diag(alpha1) @ W2).astype(bf)
    w2na = _blk(np.diag(1.0 - alpha1) @ W2).astype(bf)
    w34p = np.zeros((32, 2), np.float32)
    w34p[0:16, 0] = W3[:, 0] * (1.0 - alpha2)
    w34p[16:32, 1] = W3[:, 0] * (1.0 - alpha2)
    w34 = np.tile(w34p, (4, 1)).astype(bf)
    colsb = np.zeros((128, 8), np.float32)
    colsb[:, 0] = np.tile(inv1, 2)
    colsb[:, 1] = np.tile(-mean1 * inv1, 2)
    colsb[:, 2] = np.tile(alpha1 / (1.0 - alpha1), 2)
    colsb[:, 3] = np.tile(alpha1, 2)
    colsb[:, 4] = np.tile(inv2, 8)
    colsb[:, 5] = np.tile(-mean2 * inv2, 8)
    colsb[:, 6] = np.tile(alpha2 / (1.0 - alpha2), 8)
    colsb[:, 7] = np.tile(alpha2, 8)
    return Wq, wk2, wqk2, w2a, w2na, w34, colsb


def kernel(query_emb, key_emb, seq_length, W1, alpha1, mean1, var1,
           W2, alpha2, mean2, var2, W3):
    (Wq, wk2, wqk2, w2a, w2na, w34, colsb) = _prep_consts(
        np.asarray(W1, np.float32), np.asarray(alpha1, np.float32),
        np.asarray(mean1, np.float32), np.asarray(var1, np.float32),
        np.asarray(W2, np.float32), np.asarray(alpha2, np.float32),
        np.asarray(mean2, np.float32), np.asarray(var2, np.float32),
        np.asarray(W3, np.float32))
    q = np.asarray(query_emb, np.float32)
    k = np.asarray(key_emb, np.float32)
    sl = np.asarray(seq_length).reshape(-1).astype(np.int64)

    qW = (q @ Wq).astype(np.float32)  # [B, 64]

    order = np.argsort(sl, kind="stable")
    shards = [order[c::NCORES] for c in range(NCORES)]

    sched = []
    for g in range(NGROUPS):
        mx = max(int(sl[shards[c][16 * g : 16 * g + 16]].max()) for c in range(NCORES))
        sched.append((min(128, _ceil16(mx)), _ceil16(max(0, mx - 128))))
    sched = tuple(sched)

    if sched not in _CACHE:
        _CACHE[sched] = _build(sched)
    nc = _CACHE[sched]
    npair = BLOC // 2

    t_full = np.arange(128)[:, None]
    t_part = np.arange(128)[:, None] + 128

    in_maps = []
    for c in range(NCORES):
        rows = shards[c]
        slc = sl[rows]
        qs = q[rows]          # [256, 64]
        qWs = qW[rows]        # [256, 64]

        qp_t = np.zeros((128, npair), np.float32)
        qp_t[0:64] = qs[0::2].T
        qp_t[64:128] = qs[1::2].T

        qw1_t = np.zeros((1, 128 * npair), np.float32)
        qw1_r = qw1_t.reshape(npair, 2, 64)
        qw1_r[:, 0, :] = qWs[0::2]
        qw1_r[:, 1, :] = qWs[1::2]

        mk = np.zeros((128, 32 * NGROUPS), np.float32)
        for g in range(NGROUPS):
            sg_ = slc[16 * g : 16 * g + 16]
            full = (t_full < sg_[None, :]).astype(np.float32)   # [128, 16]
            part = (t_part < sg_[None, :]).astype(np.float32)
            mk[:, 32 * g + 0 : 32 * g + 32 : 4] = full[:, 0::2]
            mk[:, 32 * g + 1 : 32 * g + 32 : 4] = full[:, 1::2]
            mk[:, 32 * g + 2 : 32 * g + 32 : 4] = part[:, 0::2]
            mk[:, 32 * g + 3 : 32 * g + 32 : 4] = part[:, 1::2]

        ks_ = k[rows]  # [256, 200, 64]
        keyt_h = np.zeros((4, 256, 64, D), np.float32)
        for b_ in range(4):
            keyt_h[b_, 0:S] = ks_[64 * b_ : 64 * b_ + 64].transpose(1, 0, 2)
        cbf_h = np.concatenate(
            [mk, wk2, wqk2, w2a, w2na, w34], axis=1).astype(bf)
        n_wqh = max(1, (NGROUPS - WQH_GE) * 8)
        if WQH_GE < NGROUPS:
            wq_l = qp_t[:, 8 * WQH_GE :]                      # [128, n_late]
            wqk_f = np.asarray(wqk2, np.float32)
            wqh_h = (wq_l[:, :, None] * wqk_f[:, None, :]).astype(bf)
            wqh_h = wqh_h.reshape(128, (NGROUPS - WQH_GE) * 8 * 128)
        else:
            wqh_h = np.zeros((128, n_wqh * 128), bf)
        cf32_h = np.concatenate([qp_t, colsb], axis=1).astype(np.float32)
        in_maps.append({
            "keyt": keyt_h.reshape(4 * 256 * 64, D).astype(bf),
            "qp": qp_t,
            "qw1": qw1_t.astype(bf),
            "maskd": mk.astype(bf),
            "wk2": wk2, "wqk2": wqk2, "w2a": w2a, "w2na": w2na,
            "w34": w34, "colsb": colsb,
            "cbf": cbf_h, "cf32": cf32_h, "wqh": wqh_h,
        })

    res = run_bass_kernel_spmd(nc, in_maps, list(range(NCORES)), trace=TRACE)
    global LAST_RESULT
    LAST_RESULT = res

    out_full = np.zeros((B, D), np.float32)
    for c in range(NCORES):
        out_full[shards[c]] = np.asarray(res.results[c]["out"], np.float32).T
    return out_full
